# revision 11
# baseline (speedup 1.0000x reference)
"""TopK autoencoder (encode -> top-256 by |.| -> mask -> decode) on 8 TRN2 cores.

Data-parallel over batch (512 rows/core), all matmuls fp16 (1 cyc/row).
Selection is exact: candidates, threshold and mask all use the same fp32
feat values (psum copy == DRAM spill bits).

Per core:
  - encode (row-major): feat[r, f] accumulated in PSUM with xT tiles
    stationary and fp16 W streaming; W read fp32 (plain HWDGE, 2KB runs),
    converted to fp16 on ACT, fp16 copy written to wh16 DRAM (blocked
    [fc][128, dc, 512] so the write is one contiguous 2 MiB burst) for
    decode. feat spilled fp32 row-major. Top-16 candidates per
    512-feature chunk extracted from PSUM squares (max8 + match_replace
    + max8). Candidate buffer compressed to a running top-256 halfway so
    only the final 256th-largest extraction is exposed.
  - mask + decode: feat streams back row-major; enc = (feat^2 >= thr) *
    feat via one scalar_tensor_tensor per chunk (thr per-partition);
    enc tiles PE-transposed (post-selection, values only) into fp16
    encT feeding decode as stationary vs DMA-transposed fp16 W.T tiles;
    x_hat accumulates in PSUM over groups of 8 feature tiles.
"""

import numpy as np

B, D, F, K = 4096, 2048, 32768, 256
NCORES = 8
BSH = B // NCORES  # 512 rows per core
RT = BSH // 128    # 4 row tiles
DC = D // 128      # 16 contraction chunks (encode)
FC = F // 512      # 64 feature chunks (candidate granularity)
KC = F // 128      # 256 feature tiles (128-wide)
GK = 8             # decode feature tiles per group (1024 features)
NG = KC // GK      # 32 decode groups

_CACHE = {}
LAST_RESULTS = None


def _build():
    from concourse import bacc, mybir, tile, masks

    f32 = mybir.dt.float32
    f16 = mybir.dt.float16
    ge = mybir.AluOpType.is_ge
    mult = mybir.AluOpType.mult

    nc = bacc.Bacc(trn_type="TRN2", target_bir_lowering=False, debug=False)
    x_in = nc.dram_tensor("x", [RT, 128, D], f32, kind="ExternalInput").ap()
    w_in = nc.dram_tensor("W", [D, F], f32, kind="ExternalInput").ap()
    b_in = nc.dram_tensor("b", [1, D], f32, kind="ExternalInput").ap()
    xhat_out = nc.dram_tensor("xhat", [RT, 128, D], f32, kind="ExternalOutput").ap()

    wh_dram = nc.dram_tensor("wh16", [D, F], f16).ap()
    feat_dram = nc.dram_tensor("feat", [RT, 128, F], f32).ap()

    with tile.TileContext(nc) as tc:
        with tc.tile_pool(name="glob", bufs=1) as gp:
            ident = gp.tile([128, 128], f32, tag="ident")
            masks.make_identity(nc, ident[:])
            bfull = gp.tile([128, D], f32, tag="bfull")
            cands = [
                gp.tile([128, 768], f32, tag=f"cand{rt}", name=f"cand{rt}")
                for rt in range(RT)
            ]
            thrs = [
                gp.tile([128, 1], f32, tag=f"thr{rt}", name=f"thr{rt}")
                for rt in range(RT)
            ]
            xaccs = [
                gp.tile([128, D], f32, tag=f"xacc{rt}", name=f"xacc{rt}")
                for rt in range(RT)
            ]

            with tc.tile_pool(name="pAB", bufs=1) as pAB:
                xT = pAB.tile([128, DC, BSH], f16, tag="xT")

                # ---- phase A: load b/x, subtract b, transpose x to fp16 xT ----
                with (
                    tc.tile_pool(name="pA", bufs=2) as pA,
                    tc.tile_pool(name="psA", bufs=4, space="PSUM") as psA,
                ):
                    bt = pA.tile([1, D], f32, tag="bt")
                    nc.sync.dma_start(bt[:], b_in)
                    nc.gpsimd.partition_broadcast(bfull[:], bt[:])
                    for rt in range(RT):
                        xrow = pA.tile([128, D], f32, tag="xrow")
                        nc.sync.dma_start(xrow[:], x_in[rt])
                        nc.vector.tensor_sub(xrow[:], xrow[:], bfull[:])
                        for dc in range(DC):
                            pt0 = psA.tile([128, 128], f32, tag="pt0")
                            nc.tensor.transpose(
                                pt0[:], xrow[:, dc * 128 : (dc + 1) * 128], ident[:]
                            )
                            nc.vector.tensor_copy(
                                xT[:, dc, rt * 128 : (rt + 1) * 128], pt0[:]
                            )

                # ---- phase B: encode row-major, spill, candidates ----
                with (
                    tc.tile_pool(name="pBr", bufs=2) as pBr,
                    tc.tile_pool(name="pBw", bufs=2) as pBw,
                    tc.tile_pool(name="pBf", bufs=3) as pBf,
                    tc.tile_pool(name="pBs", bufs=2) as pBs,
                    tc.tile_pool(name="pBm", bufs=2) as pBm,
                    tc.tile_pool(name="psB", bufs=6, space="PSUM") as psB,
                ):
                    for fc in range(FC):
                        fcs = fc * 512
                        wsb = pBw.tile([128, DC, 512], f16, tag="wsb")
                        for h in range(2):
                            w32 = pBr.tile([128, DC // 2, 512], f32, tag="w32")
                            nc.sync.dma_start(
                                w32[:],
                                w_in[h * 1024 : (h + 1) * 1024, fcs : fcs + 512]
                                .rearrange("(dc p) n -> p dc n", p=128),
                            )
                            nc.scalar.copy(
                                wsb[:, h * (DC // 2) : (h + 1) * (DC // 2), :], w32[:]
                            )
                        nc.scalar.dma_start(
                            wh_dram[:, fcs : fcs + 512].rearrange(
                                "(dc p) n -> p dc n", p=128
                            ),
                            wsb[:],
                        )
                        for rt in range(RT):
                            ps = psB.tile([128, 512], f32, tag="ps")
                            for dc in range(DC):
                                nc.tensor.matmul(
                                    ps[:],
                                    xT[:, dc, rt * 128 : (rt + 1) * 128],
                                    wsb[:, dc, :],
                                    start=(dc == 0),
                                    stop=(dc == DC - 1),
                                )
                            fsb = pBf.tile([128, 512], f32, tag="fsb")
                            nc.scalar.copy(fsb[:], ps[:])
                            nc.sync.dma_start(
                                feat_dram[rt][:, fcs : fcs + 512], fsb[:]
                            )
                            sq = pBs.tile([128, 512], f32, tag="sq")
                            nc.scalar.square(sq[:], ps[:])
                            idx = 256 + (fc % 32) * 16
                            c8 = cands[rt][:, idx : idx + 8]
                            nc.vector.max(c8, sq[:])
                            nc.vector.match_replace(sq[:], c8, sq[:], -1.0)
                            nc.vector.max(cands[rt][:, idx + 8 : idx + 16], sq[:])
                        if fc == 31:
                            # compress first 32 fc's candidates into kept[0:256]
                            for rt in range(RT):
                                m8 = pBm.tile([128, 8], f32, tag="m8c", name=f"m8c{rt}")
                                for r in range(K // 8):
                                    nc.vector.max(m8[:], cands[rt][:, 256:768])
                                    nc.vector.match_replace(
                                        cands[rt][:, 256:768],
                                        m8[:],
                                        cands[rt][:, 256:768],
                                        -1.0,
                                    )
                                    nc.vector.tensor_copy(
                                        cands[rt][:, r * 8 : (r + 1) * 8], m8[:]
                                    )

            # ---- phase C: final 256th-largest extraction -> thrs ----
            with tc.tile_pool(name="pC", bufs=2) as pC:
                for rt in range(RT):
                    m8 = pC.tile([128, 8], f32, tag="m8", name=f"m8_{rt}")
                    for r in range(K // 8):
                        nc.vector.max(m8[:], cands[rt][:])
                        if r < K // 8 - 1:
                            nc.vector.match_replace(
                                cands[rt][:], m8[:], cands[rt][:], -1.0
                            )
                    nc.vector.tensor_copy(thrs[rt][:], m8[:, 7:8])

            # ---- phase D: mask + transpose enc + decode ----
            with (
                tc.tile_pool(name="pDw", bufs=2) as pDw,
                tc.tile_pool(name="pDe", bufs=16) as pDe,
                tc.tile_pool(name="pDf", bufs=3) as pDf,
                tc.tile_pool(name="pDs", bufs=2) as pDs,
                tc.tile_pool(name="pDm", bufs=8) as pDm,
                tc.tile_pool(name="psE", bufs=2, space="PSUM") as psE,
                tc.tile_pool(name="psD", bufs=3, space="PSUM") as psD,
            ):
                for g in range(NG):
                    gfs = g * 1024
                    wtg = pDw.tile([128, GK, D], f16, tag="wtg")
                    for i in range(GK):
                        kc = g * GK + i
                        nc.sync.dma_start(
                            wtg[:, i],
                            wh_dram[:, kc * 128 : (kc + 1) * 128],
                            transpose=True,
                        )
                    enchs = []
                    for rt in range(RT):
                        fch = pDf.tile([128, 1024], f32, tag="fch")
                        nc.scalar.dma_start(
                            fch[:], feat_dram[rt][:, gfs : gfs + 1024]
                        )
                        sqc = pDs.tile([128, 1024], f32, tag="sqc")
                        nc.scalar.square(sqc[:], fch[:])
                        ench = pDm.tile([128, 1024], f32, tag="ench")
                        nc.vector.scalar_tensor_tensor(
                            out=ench[:],
                            in0=sqc[:],
                            scalar=thrs[rt][:],
                            in1=fch[:],
                            op0=ge,
                            op1=mult,
                        )
                        enchs.append(ench)
                    encT = []
                    for i in range(GK):
                        pse = psE.tile([128, 512], f32, tag="pse")
                        for rt in range(RT):
                            nc.tensor.transpose(
                                pse[:, rt * 128 : (rt + 1) * 128],
                                enchs[rt][:, i * 128 : (i + 1) * 128],
                                ident[:],
                            )
                        et = pDe.tile([128, BSH], f16, tag="et")
                        nc.vector.tensor_copy(et[:], pse[:])
                        encT.append(et)
                    for rt in range(RT):
                        for dh in range(2):
                            px = psD.tile([128, 1024], f32, tag="px")
                            for i in range(GK):
                                lhsT = encT[i][:, rt * 128 : (rt + 1) * 128]
                                for ds in range(2):
                                    nc.tensor.matmul(
                                        px[:, ds * 512 : (ds + 1) * 512],
                                        lhsT,
                                        wtg[:, i, dh * 1024 + ds * 512 : dh * 1024 + (ds + 1) * 512],
                                        start=(i == 0),
                                        stop=(i == GK - 1),
                                    )
                            xa = xaccs[rt][:, dh * 1024 : (dh + 1) * 1024]
                            if g == 0:
                                nc.scalar.copy(xa, px[:])
                            else:
                                nc.vector.tensor_add(xa, xa, px[:])

            # ---- phase E: + b_dec, write out ----
            for rt in range(RT):
                nc.vector.tensor_add(xaccs[rt][:], xaccs[rt][:], bfull[:])
                nc.sync.dma_start(xhat_out[rt], xaccs[rt][:])

    nc.compile()
    return nc


def kernel(x, W, b_dec, trace=False):
    global LAST_RESULTS
    from concourse.bass_utils import run_bass_kernel_spmd

    if "nc" not in _CACHE:
        _CACHE["nc"] = _build()
    nc = _CACHE["nc"]

    x = np.ascontiguousarray(np.asarray(x, dtype=np.float32))
    W = np.ascontiguousarray(np.asarray(W, dtype=np.float32))
    b = np.ascontiguousarray(np.asarray(b_dec, dtype=np.float32)).reshape(1, D)

    in_maps = []
    for c in range(NCORES):
        xs = x[c * BSH : (c + 1) * BSH].reshape(RT, 128, D)
        in_maps.append({"x": xs, "W": W, "b": b})

    kwargs = {}
    if trace:
        kwargs = dict(trace=True, trace_cores=[0])
    res = run_bass_kernel_spmd(nc, in_maps, core_ids=list(range(NCORES)), **kwargs)
    LAST_RESULTS = res
    out = np.concatenate(
        [res.results[c]["xhat"].reshape(BSH, D) for c in range(NCORES)], axis=0
    )
    return out


# revision 21
# speedup vs baseline: 1.1864x; 1.1864x over previous
"""TopK autoencoder (encode -> top-256 by |.| -> mask -> decode) on 8 TRN2 cores.

Data-parallel over batch (512 rows/core). Encode matmuls run as fp32r
(1 cyc/row, ~12-bit effective input precision -- 2x better boundary
accuracy than fp16); decode matmuls fp16. Selection is exact w.r.t. the
fp32 PSUM feat values (candidates, threshold and mask all read the same
bits).

Per core:
  - encode (row-major): feat[r, f] in PSUM with xT tiles stationary and
    W streaming, both bitcast to fp32r. W read fp32 once (2KB runs); an
    fp16 copy goes to wh16 DRAM in blocked layout [fc][p, dc, n] so the
    write is one contiguous 2 MiB burst and the decode DMA-transpose
    reads merge rows at stride 1KB (16x fewer descriptors than the
    plain [D, F] layout). feat spilled fp32 row-major. Top-16
    candidates per 512-feature chunk from PSUM squares; candidate
    buffer compressed to a running top-256 halfway so only the final
    extraction is exposed.
  - mask + decode: feat streams back row-major; enc = (feat^2 >= thr) *
    feat via one scalar_tensor_tensor per chunk; enc tiles PE-transposed
    (post-selection) into fp16 encT feeding decode as stationary vs
    DMA-transposed fp16 W.T tiles. The blocked wh16 makes decode's
    free dim a permuted d' = (d%128)*16 + d//128 order; one strided
    DVE tensor_tensor per row tile un-permutes while adding b_dec.
"""

import numpy as np

B, D, F, K = 4096, 2048, 32768, 256
NCORES = 8
BSH = B // NCORES  # 512 rows per core
RT = BSH // 128    # 4 row tiles
DC = D // 128      # 16 contraction chunks (encode)
FC = F // 512      # 64 feature chunks (candidate granularity)
KC = F // 128      # 256 feature tiles (128-wide)
GK = 8             # decode feature tiles per group (1024 features)
NG = KC // GK      # 32 decode groups

_CACHE = {}
LAST_RESULTS = None


def _build(debug=False):
    from concourse import bacc, mybir, tile, masks

    f32 = mybir.dt.float32
    f32r = mybir.dt.float32r
    f16 = mybir.dt.float16
    ge = mybir.AluOpType.is_ge
    mult = mybir.AluOpType.mult
    add = mybir.AluOpType.add

    nc = bacc.Bacc(trn_type="TRN2", target_bir_lowering=False, debug=False)
    x_in = nc.dram_tensor("x", [RT, 128, D], f32, kind="ExternalInput").ap()
    w_in = nc.dram_tensor("W", [D, F], f32r, kind="ExternalInput").ap()
    b_in = nc.dram_tensor("b", [1, D], f32, kind="ExternalInput").ap()
    xhat_out = nc.dram_tensor("xhat", [RT, 128, D], f32, kind="ExternalOutput").ap()

    # blocked fp16 W copy: [fc][r', n] with r' = p*16 + dc, d = dc*128 + p,
    # f = fc*512 + n  (rows merged so decode's transposed read is a plain slice)
    wh_dram = nc.dram_tensor("wh16", [FC, 2048, 512], f16).ap()
    feat_dram = nc.dram_tensor("feat", [RT, 128, F], f32).ap()
    if debug:
        dbg_feat = nc.dram_tensor("dbg_feat", [128, 1024], f32, kind="ExternalOutput").ap()
        dbg_wh = nc.dram_tensor("dbg_wh", [2048, 512], f16, kind="ExternalOutput").ap()
        dbg_thr = nc.dram_tensor("dbg_thr", [RT, 128, 1], f32, kind="ExternalOutput").ap()
        dbg_wt = nc.dram_tensor("dbg_wt", [128, D], f16, kind="ExternalOutput").ap()
        dbg_et = nc.dram_tensor("dbg_et", [128, BSH], f16, kind="ExternalOutput").ap()

    with tile.TileContext(nc) as tc:
        with tc.tile_pool(name="glob", bufs=1) as gp:
            ident = gp.tile([128, 128], f32, tag="ident")
            masks.make_identity(nc, ident[:])
            bfull = gp.tile([128, D], f32, tag="bfull")
            cands = [
                gp.tile([128, 768], f32, tag=f"cand{rt}", name=f"cand{rt}")
                for rt in range(RT)
            ]
            thrs = [
                gp.tile([128, 1], f32, tag=f"thr{rt}", name=f"thr{rt}")
                for rt in range(RT)
            ]
            xaccs = [
                gp.tile([128, D], f32, tag=f"xacc{rt}", name=f"xacc{rt}")
                for rt in range(RT)
            ]

            with tc.tile_pool(name="pAB", bufs=1) as pAB:
                xT = pAB.tile([128, DC, BSH], f32r, tag="xT")

                # ---- phase A: load b/x, subtract b, transpose x ----
                with (
                    tc.tile_pool(name="pA", bufs=2) as pA,
                    tc.tile_pool(name="psA", bufs=4, space="PSUM") as psA,
                ):
                    bt = pA.tile([1, D], f32, tag="bt")
                    nc.sync.dma_start(bt[:], b_in)
                    nc.gpsimd.partition_broadcast(bfull[:], bt[:])
                    for rt in range(RT):
                        xrow = pA.tile([128, D], f32, tag="xrow")
                        nc.sync.dma_start(xrow[:], x_in[rt])
                        nc.vector.tensor_sub(xrow[:], xrow[:], bfull[:])
                        for dc in range(DC):
                            pt0 = psA.tile([128, 128], f32, tag="pt0")
                            nc.tensor.transpose(
                                pt0[:], xrow[:, dc * 128 : (dc + 1) * 128], ident[:]
                            )
                            nc.vector.tensor_copy(
                                xT[:, dc, rt * 128 : (rt + 1) * 128], pt0[:]
                            )

                # ---- phase B: encode row-major (fp32r), spill, candidates ----
                with (
                    tc.tile_pool(name="pBw", bufs=2) as pBw,
                    tc.tile_pool(name="pBh", bufs=2) as pBh,
                    tc.tile_pool(name="pBf", bufs=3) as pBf,
                    tc.tile_pool(name="pBs", bufs=2) as pBs,
                    tc.tile_pool(name="pBm", bufs=2) as pBm,
                    tc.tile_pool(name="psB", bufs=6, space="PSUM") as psB,
                ):
                    for fc in range(FC):
                        fcs = fc * 512
                        wsb = pBw.tile([128, DC, 512], f32r, tag="wsb")
                        for h in range(2):
                            nc.sync.dma_start(
                                wsb[:, h * (DC // 2) : (h + 1) * (DC // 2), :],
                                w_in[h * 1024 : (h + 1) * 1024, fcs : fcs + 512]
                                .rearrange("(dc p) n -> p dc n", p=128),
                            )
                        wsb16 = pBh.tile([128, DC, 512], f16, tag="wsb16")
                        nc.scalar.copy(wsb16[:], wsb[:].bitcast(f32))
                        nc.scalar.dma_start(
                            wh_dram[fc].rearrange("(p dc) n -> p dc n", p=128),
                            wsb16[:],
                        )
                        for rt in range(RT):
                            ps = psB.tile([128, 512], f32, tag="ps")
                            for dc in range(DC):
                                nc.tensor.matmul(
                                    ps[:],
                                    xT[:, dc, rt * 128 : (rt + 1) * 128],
                                    wsb[:, dc, :],
                                    start=(dc == 0),
                                    stop=(dc == DC - 1),
                                )
                            fsb = pBf.tile([128, 512], f32, tag="fsb")
                            nc.scalar.copy(fsb[:], ps[:])
                            nc.scalar.dma_start(
                                feat_dram[rt][:, fcs : fcs + 512], fsb[:]
                            )
                            sq = pBs.tile([128, 512], f32, tag="sq")
                            nc.scalar.square(sq[:], ps[:])
                            idx = 256 + (fc % 32) * 16
                            c8 = cands[rt][:, idx : idx + 8]
                            nc.vector.max(c8, sq[:])
                            nc.vector.match_replace(sq[:], c8, sq[:], -1.0)
                            nc.vector.max(cands[rt][:, idx + 8 : idx + 16], sq[:])
                        if fc == 31:
                            # compress first 32 fc's candidates into kept[0:256]
                            for rt in range(RT):
                                m8 = pBm.tile([128, 8], f32, tag="m8c", name=f"m8c{rt}")
                                for r in range(K // 8):
                                    nc.vector.max(m8[:], cands[rt][:, 256:768])
                                    nc.vector.match_replace(
                                        cands[rt][:, 256:768],
                                        m8[:],
                                        cands[rt][:, 256:768],
                                        -1.0,
                                    )
                                    nc.vector.tensor_copy(
                                        cands[rt][:, r * 8 : (r + 1) * 8], m8[:]
                                    )

            # ---- phase C: final 256th-largest extraction -> thrs ----
            with tc.tile_pool(name="pC", bufs=2) as pC:
                for rt in range(RT):
                    m8 = pC.tile([128, 8], f32, tag="m8", name=f"m8_{rt}")
                    for r in range(K // 8):
                        nc.vector.max(m8[:], cands[rt][:])
                        if r < K // 8 - 1:
                            nc.vector.match_replace(
                                cands[rt][:], m8[:], cands[rt][:], -1.0
                            )
                    nc.vector.tensor_copy(thrs[rt][:], m8[:, 7:8])
                    if debug:
                        nc.sync.dma_start(dbg_thr[rt], thrs[rt][:])

            # ---- phase D: mask + transpose enc + decode ----
            with (
                tc.tile_pool(name="pDw", bufs=4) as pDw,
                tc.tile_pool(name="pDe", bufs=16) as pDe,
                tc.tile_pool(name="pDf", bufs=3) as pDf,
                tc.tile_pool(name="pDs", bufs=2) as pDs,
                tc.tile_pool(name="pDm", bufs=8) as pDm,
                tc.tile_pool(name="psE", bufs=2, space="PSUM") as psE,
                tc.tile_pool(name="psD", bufs=3, space="PSUM") as psD,
            ):
                for g in range(NG):
                    gfs = g * 1024
                    wt4s = []
                    for half in range(2):
                        wt4 = pDw.tile([128, 4, D], f16, tag="wt4", name=f"wt4_{half}")
                        for ii in range(4):
                            kc = g * GK + half * 4 + ii
                            nc.sync.dma_start(
                                wt4[:, ii],
                                wh_dram[kc // 4][:, (kc % 4) * 128 : (kc % 4 + 1) * 128],
                                transpose=True,
                            )
                        wt4s.append(wt4)
                    enchs = []
                    for rt in range(RT):
                        fch = pDf.tile([128, 1024], f32, tag="fch")
                        nc.scalar.dma_start(
                            fch[:], feat_dram[rt][:, gfs : gfs + 1024]
                        )
                        sqc = pDs.tile([128, 1024], f32, tag="sqc")
                        nc.scalar.square(sqc[:], fch[:])
                        ench = pDm.tile([128, 1024], f32, tag="ench")
                        nc.vector.scalar_tensor_tensor(
                            out=ench[:],
                            in0=sqc[:],
                            scalar=thrs[rt][:],
                            in1=fch[:],
                            op0=ge,
                            op1=mult,
                        )
                        enchs.append(ench)
                    encT = []
                    for i in range(GK):
                        pse = psE.tile([128, 512], f32, tag="pse")
                        for rt in range(RT):
                            nc.tensor.transpose(
                                pse[:, rt * 128 : (rt + 1) * 128],
                                enchs[rt][:, i * 128 : (i + 1) * 128],
                                ident[:],
                            )
                        et = pDe.tile([128, BSH], f16, tag="et")
                        nc.vector.tensor_copy(et[:], pse[:])
                        encT.append(et)
                        if debug and g == 0 and i == 0:
                            nc.sync.dma_start(dbg_et, et[:])
                            nc.sync.dma_start(dbg_wt, wt4s[0][:, 0])
                    for rt in range(RT):
                        for dh in range(2):
                            px = psD.tile([128, 1024], f32, tag="px")
                            for i in range(GK):
                                lhsT = encT[i][:, rt * 128 : (rt + 1) * 128]
                                wt4 = wt4s[i // 4]
                                for ds in range(2):
                                    nc.tensor.matmul(
                                        px[:, ds * 512 : (ds + 1) * 512],
                                        lhsT,
                                        wt4[:, i % 4, dh * 1024 + ds * 512 : dh * 1024 + (ds + 1) * 512],
                                        start=(i == 0),
                                        stop=(i == GK - 1),
                                    )
                            xa = xaccs[rt][:, dh * 1024 : (dh + 1) * 1024]
                            if g == 0:
                                nc.scalar.copy(xa, px[:])
                            else:
                                nc.vector.tensor_add(xa, xa, px[:])

            if debug:
                nc.sync.dma_start(dbg_feat, feat_dram[0][:, 0:1024])
                nc.sync.dma_start(dbg_wh, wh_dram[0])

            # ---- phase E: un-permute d' -> d, + b_dec, write out ----
            with tc.tile_pool(name="pE", bufs=2) as pE:
                for rt in range(RT):
                    xout = pE.tile([128, D], f32, tag="xout")
                    nc.vector.tensor_tensor(
                        xout[:],
                        xaccs[rt][:].rearrange("q (p dc) -> q dc p", p=128),
                        bfull[:],
                        add,
                    )
                    nc.sync.dma_start(xhat_out[rt], xout[:])

    nc.compile()
    return nc


def kernel(x, W, b_dec, trace=False):
    global LAST_RESULTS
    from concourse.bass_utils import run_bass_kernel_spmd

    if "nc" not in _CACHE:
        # debug=True keeps five tiny staging DMAs that also serialize the
        # inter-phase schedule; the non-debug schedule has a mis-ordering
        # that corrupts multi-core runs.
        _CACHE["nc"] = _build(debug=True)
    nc = _CACHE["nc"]

    x = np.ascontiguousarray(np.asarray(x, dtype=np.float32))
    W = np.ascontiguousarray(np.asarray(W, dtype=np.float32))
    b = np.ascontiguousarray(np.asarray(b_dec, dtype=np.float32)).reshape(1, D)

    in_maps = []
    for c in range(NCORES):
        xs = x[c * BSH : (c + 1) * BSH].reshape(RT, 128, D)
        in_maps.append({"x": xs, "W": W, "b": b})

    kwargs = {}
    if trace:
        kwargs = dict(trace=True, trace_cores=[0])
    res = run_bass_kernel_spmd(nc, in_maps, core_ids=list(range(NCORES)), **kwargs)
    LAST_RESULTS = res
    out = np.concatenate(
        [res.results[c]["xhat"].reshape(BSH, D) for c in range(NCORES)], axis=0
    )
    return out
